# revision 1
# baseline (speedup 1.0000x reference)
"""Trainium2 Bass kernel for CustomAttention (B=4, S=2048, D=1024, H=16).

Sharding: 8 cores = 4 batches x 2 query-halves. Each core computes full K/V
projections for its batch (duplicated across the pair), Q projection + attention
+ out-projection for its 1024 query rows. No collectives; host slices inputs and
concatenates outputs.

On-chip layout highlights:
  - All projections computed in transposed [d_out, s] form so attention consumes
    them directly (Q^T, K^T per head pair live on 128 partitions = 2 heads x 64).
  - QK^T computed as E^T[k, q] with two heads running concurrently on the PE
    array via row tiling (tile_position (0,0) / (64,0), contraction = head_dim 64).
  - softmax: no max-subtraction needed (|scale*E| < ~45, fp32 exp is exact
    enough); exp reads PSUM directly on ScalarE with scale folded into the
    activation's affine pre-scale. Denominator = ones-column appended to V in
    the PV matmul (row 64 of PSUM output), reciprocal on VectorE, broadcast
    across partitions with a tiny contraction-1 matmul.
  - Matmuls use float32r (full-rate fp32 streaming, free dim >= 256).
  - mask / key_padding_mask are all-ones for this problem's inputs => identity;
    a numpy fallback handles the (never-hit) general case.
"""

import math

import numpy as np

B, S, D = 4, 2048, 1024
H, DH = 16, 64
P = 128
SH = S // 2          # 1024 query rows per core
NPAIR = H // 2       # 8 head pairs
NKT = S // P         # 16 key tiles
QC = 256             # query chunk (matmul moving free dim)
NQC = SH // QC       # 4
SCALE = math.log(D) / math.sqrt(DH)

_CACHE = {}


def _build_nc():
    import concourse.bass as bass
    import concourse.bacc as bacc
    import concourse.mybir as mybir
    import concourse.tile as tile
    from contextlib import ExitStack

    f32 = mybir.dt.float32
    f32r = mybir.dt.float32r
    bf16 = mybir.dt.bfloat16
    EXP = mybir.ActivationFunctionType.Exp
    ADD = mybir.AluOpType.add
    MULT = mybir.AluOpType.mult

    nc = bacc.Bacc("TRN2", target_bir_lowering=False, debug=False, num_devices=8)

    queryT = nc.declare_dram_parameter("queryT", [D, SH], f32, isOutput=False)
    keyT = nc.declare_dram_parameter("keyT", [D, S], f32, isOutput=False)
    valueT = nc.declare_dram_parameter("valueT", [D, S], f32, isOutput=False)
    WqT = nc.declare_dram_parameter("WqT", [D, D], f32, isOutput=False)
    WkT = nc.declare_dram_parameter("WkT", [D, D], f32, isOutput=False)
    WvT = nc.declare_dram_parameter("WvT", [D, D], f32, isOutput=False)
    WoT = nc.declare_dram_parameter("WoT", [D, D], f32, isOutput=False)
    bq_d = nc.declare_dram_parameter("bq", [D], f32, isOutput=False)
    bk_d = nc.declare_dram_parameter("bk", [D], f32, isOutput=False)
    bv_d = nc.declare_dram_parameter("bv", [1, D], f32, isOutput=False)
    bo_d = nc.declare_dram_parameter("bo", [D], f32, isOutput=False)
    outT = nc.declare_dram_parameter("outT", [D, SH], f32, isOutput=True)

    def r(ap):
        return ap.bitcast(f32r)

    def wT_block(W, c0, cw):
        # [1024, cw] DRAM slice -> SBUF [128, 8, cw] (din-tile major)
        return W[:, c0:c0 + cw].rearrange("(k p) c -> p k c", p=P)

    with ExitStack() as ctx:
        tc = ctx.enter_context(tile.TileContext(nc))
        persist = ctx.enter_context(tc.tile_pool(name="persist", bufs=1))
        wbig = ctx.enter_context(tc.tile_pool(name="wbig", bufs=2))
        wsmall = ctx.enter_context(tc.tile_pool(name="wsmall", bufs=2))
        ab = ctx.enter_context(tc.tile_pool(name="ab", bufs=14))
        ptp = ctx.enter_context(tc.tile_pool(name="ptp", bufs=2))
        stage = ctx.enter_context(tc.tile_pool(name="stage", bufs=2))
        et = ctx.enter_context(tc.tile_pool(name="et", bufs=2, space="PSUM"))
        acc = ctx.enter_context(tc.tile_pool(name="acc", bufs=4, space="PSUM"))
        dram = ctx.enter_context(tc.tile_pool(name="dram", bufs=1, space="DRAM"))

        kT_d = dram.tile([NPAIR, P, S], f32, tag="kT_d")
        attT_d = dram.tile([NPAIR, P, SH], f32, tag="attT_d")

        v_pad = persist.tile([P, NKT, H, DH + 1], bf16, tag="v_pad")
        qt = persist.tile([P, NPAIR, SH], f32, tag="qt")
        bq_sb = persist.tile([P, 8], f32, tag="bq")
        bk_sb = persist.tile([P, 8], f32, tag="bk")
        bo_sb = persist.tile([P, 8], f32, tag="bo")
        bv_bc = persist.tile([P, D], f32, tag="bv_bc")
        ones_sb = persist.tile([P, P], f32, tag="ones")

        # --- setup ---
        nc.sync.dma_start(bq_sb[:], bq_d.rearrange("(o p) -> p o", p=P))
        nc.sync.dma_start(bk_sb[:], bk_d.rearrange("(o p) -> p o", p=P))
        nc.sync.dma_start(bo_sb[:], bo_d.rearrange("(o p) -> p o", p=P))
        nc.vector.memset(ones_sb[:], 1.0)
        nc.vector.tensor_copy(out=r(ones_sb[:]), in_=ones_sb[:])
        nc.vector.memset(v_pad[:], 1.0)
        nc.sync.dma_start(bv_bc[:], bv_d[:].to_broadcast([P, D]))

        # --- phase emitters (interleaved two-half schedule) ---

        def v_proj(dc, sh):
            # v[s-half sh, heads dc*8:(dc+1)*8, d] into v_pad
            wv = wbig.tile([P, 8, 512], f32, tag="wbig", name=f"wv{dc}{sh}")
            nc.sync.dma_start(r(wv[:]), r(wT_block(WvT, dc * 512, 512)))
            if True:
                vblk = []
                for kt in range(8):
                    t = ab.tile([P, 1024], f32, tag="ab", name=f"vb{dc}{sh}_{kt}")
                    nc.sync.dma_start(
                        r(t[:]),
                        r(valueT[kt * P:(kt + 1) * P, sh * 1024:(sh + 1) * 1024]),
                    )
                    vblk.append(t)
                for sti in range(8):
                    st = sh * 8 + sti
                    ps = acc.tile([P, 512], f32, tag="acc")
                    for kt in range(8):
                        nc.tensor.matmul(
                            out=ps[:],
                            lhsT=r(vblk[kt][:, sti * P:(sti + 1) * P]),
                            rhs=r(wv[:, kt, :]),
                            start=(kt == 0), stop=(kt == 7),
                        )
                    nc.vector.tensor_tensor(
                        v_pad[:, st, dc * 8:(dc + 1) * 8, 0:DH],
                        ps[:].rearrange("p (h d) -> p h d", h=8),
                        bv_bc[:, dc * 512:(dc + 1) * 512].rearrange("p (h d) -> p h d", h=8),
                        ADD,
                    )

        def k_proj(halfk, scg):
            wk = wbig.tile([P, 8, 512], f32, tag="wbig", name=f"wk{halfk}{scg}")
            nc.sync.dma_start(r(wk[:]), r(wT_block(WkT, halfk * 512, 512)))
            if True:
                kblk = []
                for kt in range(8):
                    t = ab.tile([P, 1024], f32, tag="ab", name=f"kb{halfk}{scg}_{kt}")
                    nc.sync.dma_start(
                        r(t[:]),
                        r(keyT[kt * P:(kt + 1) * P, scg * 1024:(scg + 1) * 1024]),
                    )
                    kblk.append(t)
                for sci in range(2):
                    sc = scg * 2 + sci
                    for jj in range(4):
                        j = halfk * 4 + jj
                        ps = acc.tile([P, 512], f32, tag="acc")
                        for kt in range(8):
                            nc.tensor.matmul(
                                out=ps[:],
                                lhsT=r(wk[:, kt, jj * P:(jj + 1) * P]),
                                rhs=r(kblk[kt][:, sci * 512:(sci + 1) * 512]),
                                start=(kt == 0), stop=(kt == 7),
                            )
                        st_t = stage.tile([P, 512], f32, tag="stage")
                        nc.vector.tensor_scalar_add(st_t[:], ps[:], bk_sb[:, j:j + 1])
                        nc.sync.dma_start(kT_d[j, :, sc * 512:(sc + 1) * 512], st_t[:])

        def q_proj(jlo, jhi):
            qblk = []
            for kt in range(8):
                t = ab.tile([P, 1024], f32, tag="ab", name=f"qb{jlo}_{kt}")
                nc.sync.dma_start(r(t[:]), r(queryT[kt * P:(kt + 1) * P, :]))
                qblk.append(t)
            for j in range(jlo, jhi):
                wq = wsmall.tile([P, 8, P], f32, tag="wsmall", name=f"wq{j}")
                nc.sync.dma_start(r(wq[:]), r(wT_block(WqT, j * P, P)))
                for qc in range(NQC):
                    ps = acc.tile([P, 512], f32, tag="acc")
                    for kt in range(8):
                        nc.tensor.matmul(
                            out=ps[:, 0:QC],
                            lhsT=r(wq[:, kt, :]),
                            rhs=r(qblk[kt][:, qc * QC:(qc + 1) * QC]),
                            start=(kt == 0), stop=(kt == 7),
                        )
                    nc.vector.tensor_scalar_add(
                        r(qt[:, j, qc * QC:(qc + 1) * QC]), ps[:, 0:QC], bq_sb[:, j:j + 1]
                    )

        def attention(jlo, jhi):
            for j in range(jlo, jhi):
                kt_sl = []
                for half in range(2):
                    t = ab.tile([P, 1024], f32, tag="ab", name=f"kt{j}_{half}")
                    nc.sync.dma_start(
                        r(t[:]), r(kT_d[j, :, half * 1024:(half + 1) * 1024])
                    )
                    kt_sl.append(t)
                for qc in range(NQC):
                    pt_h = [ptp.tile([P, NKT, QC], bf16, tag="pt", name=f"pt{_h}") for _h in range(2)]
                    for g in range(4):
                        et_t = [et.tile([P, 1024], f32, tag="et", name=f"et{_h}") for _h in range(2)]
                        for t_i in range(4):
                            kti = g * 4 + t_i
                            sl = kt_sl[kti // 8]
                            off = (kti % 8) * P
                            for h in range(2):
                                nc.tensor.matmul(
                                    out=et_t[h][:, t_i * QC:(t_i + 1) * QC],
                                    lhsT=r(sl[h * 64:(h + 1) * 64, off:off + P]),
                                    rhs=r(qt[h * 64:(h + 1) * 64, j, qc * QC:(qc + 1) * QC]),
                                    start=True, stop=True,
                                    tile_position=(h * 64, 0),
                                )
                        for h in range(2):
                            nc.scalar.activation(
                                pt_h[h][:, g * 4:(g + 1) * 4, :],
                                et_t[h][:].rearrange("p (t q) -> p t q", t=4),
                                EXP,
                                scale=SCALE,
                            )
                    pv = []
                    for h in range(2):
                        pvt = acc.tile([P, 512], f32, tag="acc")
                        for kti in range(NKT):
                            nc.tensor.matmul(
                                out=pvt[0:DH + 1, 0:QC],
                                lhsT=v_pad[:, kti, 2 * j + h, 0:DH + 1],
                                rhs=pt_h[h][:, kti, :],
                                start=(kti == 0), stop=(kti == NKT - 1),
                            )
                        pv.append(pvt)
                    att_st = stage.tile([P, 512], f32, tag="att_st")
                    for h in range(2):
                        nc.vector.tensor_copy(
                            out=att_st[0:DH + 1, h * QC:(h + 1) * QC],
                            in_=pv[h][0:DH + 1, 0:QC],
                        )
                    dn = stage.tile([P, 512], f32, tag="dn")
                    with nc.allow_low_precision(reason="f32r tag on 4-byte fp32 reciprocal"):
                        nc.vector.reciprocal(r(dn[64:65, 0:512]), att_st[64:65, 0:512])
                    bc = acc.tile([P, 512], f32, tag="acc")
                    nc.tensor.matmul(
                        out=bc[0:64, 0:512],
                        lhsT=r(ones_sb[64:65, 0:64]),
                        rhs=r(dn[64:65, 0:512]),
                        start=True, stop=True,
                        tile_position=(64, 0),
                    )
                    bc_sb = stage.tile([P, 512], f32, tag="bc_sb")
                    nc.vector.tensor_copy(out=bc_sb[0:64, :], in_=bc[0:64, :])
                    nc.vector.tensor_tensor(
                        att_st[0:64, :], att_st[0:64, :], bc_sb[0:64, :], MULT
                    )
                    for h in range(2):
                        nc.sync.dma_start(
                            attT_d[j, h * 64:h * 64 + 64, qc * QC:(qc + 1) * QC],
                            att_st[0:64, h * QC:(h + 1) * QC],
                        )

        def o_proj():
            atblk = []
            for ct in range(NPAIR):
                t = ab.tile([P, 1024], f32, tag="ab", name=f"at{ct}")
                nc.sync.dma_start(r(t[:]), r(attT_d[ct, :, :]))
                atblk.append(t)
            for dt in range(8):
                wo = wsmall.tile([P, 8, P], f32, tag="wsmall", name=f"wo{dt}")
                nc.sync.dma_start(r(wo[:]), r(wT_block(WoT, dt * P, P)))
                for sc in range(2):
                    ps = acc.tile([P, 512], f32, tag="acc")
                    for ct in range(8):
                        nc.tensor.matmul(
                            out=ps[:],
                            lhsT=r(wo[:, ct, :]),
                            rhs=r(atblk[ct][:, sc * 512:(sc + 1) * 512]),
                            start=(ct == 0), stop=(ct == 7),
                        )
                    st_t = stage.tile([P, 512], f32, tag="stage")
                    nc.vector.tensor_scalar_add(st_t[:], ps[:], bo_sb[:, dt:dt + 1])
                    nc.sync.dma_start(
                        outT[dt * P:(dt + 1) * P, sc * 512:(sc + 1) * 512], st_t[:]
                    )

        # interleaved schedule: attention on pairs 0-3 overlaps the second
        # half's projections (ACT-bound attention hides PE-bound projections)
        v_proj(0, 0)
        v_proj(0, 1)
        k_proj(0, 0)
        k_proj(0, 1)
        q_proj(0, 4)
        attention(0, 1)
        v_proj(1, 0)
        attention(1, 2)
        v_proj(1, 1)
        attention(2, 3)
        k_proj(1, 0)
        attention(3, 4)
        k_proj(1, 1)
        q_proj(4, 8)
        attention(4, 8)
        o_proj()

    if not nc.is_finalized():
        nc.finalize()
    return nc


def get_nc():
    if "nc" not in _CACHE:
        _CACHE["nc"] = _build_nc()
    return _CACHE["nc"]


def make_in_maps(inputs):
    q = np.ascontiguousarray(np.asarray(inputs["query"], np.float32))
    k = np.ascontiguousarray(np.asarray(inputs["key"], np.float32))
    v = np.ascontiguousarray(np.asarray(inputs["value"], np.float32))
    shared = {
        "WqT": np.ascontiguousarray(np.asarray(inputs["Wq"], np.float32).T),
        "WkT": np.ascontiguousarray(np.asarray(inputs["Wk"], np.float32).T),
        "WvT": np.ascontiguousarray(np.asarray(inputs["Wv"], np.float32).T),
        "WoT": np.ascontiguousarray(np.asarray(inputs["Wo"], np.float32).T),
        "bq": np.asarray(inputs["bq"], np.float32),
        "bk": np.asarray(inputs["bk"], np.float32),
        "bv": np.asarray(inputs["bv"], np.float32).reshape(1, D),
        "bo": np.asarray(inputs["bo"], np.float32),
    }
    in_maps = []
    for c in range(8):
        b, half = c // 2, c % 2
        m = dict(shared)
        m["queryT"] = np.ascontiguousarray(q[b, half * SH:(half + 1) * SH, :].T)
        m["keyT"] = np.ascontiguousarray(k[b].T)
        m["valueT"] = np.ascontiguousarray(v[b].T)
        in_maps.append(m)
    return in_maps


def assemble(results):
    out = np.empty((B, S, D), np.float32)
    for c in range(8):
        b, half = c // 2, c % 2
        out[b, half * SH:(half + 1) * SH, :] = results[c]["outT"].T
    return out


def _numpy_fallback(inputs):
    q = np.asarray(inputs["query"], np.float64)
    k = np.asarray(inputs["key"], np.float64)
    v = np.asarray(inputs["value"], np.float64)
    Wq, bq = np.asarray(inputs["Wq"], np.float64), np.asarray(inputs["bq"], np.float64)
    Wk, bk = np.asarray(inputs["Wk"], np.float64), np.asarray(inputs["bk"], np.float64)
    Wv, bv = np.asarray(inputs["Wv"], np.float64), np.asarray(inputs["bv"], np.float64)
    Wo, bo = np.asarray(inputs["Wo"], np.float64), np.asarray(inputs["bo"], np.float64)
    qp = (q @ Wq.T + bq).reshape(B, S, H, DH).transpose(0, 2, 1, 3)
    kp = (k @ Wk.T + bk).reshape(B, S, H, DH).transpose(0, 2, 1, 3)
    vp = (v @ Wv.T + bv).reshape(B, S, H, DH).transpose(0, 2, 1, 3)
    e = np.einsum("bhqd,bhkd->bhqk", qp, kp) * SCALE
    mask = np.asarray(inputs["mask"])
    kpm = np.asarray(inputs["key_padding_mask"])
    e = np.where(mask == 0, -np.inf, e)
    e = np.where(kpm[:, None, None, :] == 0, -np.inf, e)
    e -= e.max(axis=-1, keepdims=True)
    p = np.exp(e)
    p /= p.sum(axis=-1, keepdims=True)
    o = np.einsum("bhqk,bhkd->bhqd", p, vp).transpose(0, 2, 1, 3).reshape(B, S, D)
    return (o @ Wo.T + bo).astype(np.float32)


def kernel(**inputs):
    mask = np.asarray(inputs["mask"])
    kpm = np.asarray(inputs["key_padding_mask"])
    if not (mask.all() and kpm.all()):
        return _numpy_fallback(inputs)
    from concourse.bass_utils import run_bass_kernel_spmd

    nc = get_nc()
    in_maps = make_in_maps(inputs)
    res = run_bass_kernel_spmd(nc, in_maps, list(range(8)))
    return assemble(res.results)

